# revision 26
# baseline (speedup 1.0000x reference)
"""TRN2 Bass kernel for 16-head causal MHA (B=4, T=2048, C=2048), fp32 in/out.

Sharding: 8 cores = 4 batches x 2 head-groups (8 heads each).  Each core
computes q/k/v projections for its head group on its batch (tensor-parallel
column split of Wq/Wk/Wv), causal attention in the S^T layout, and a partial
output projection with the row slice of Wp.  The two head-group partials per
batch are summed on the host, plus the output bias.

Design (v6, bf16 datapath):
- All operand data (x^T, Wq/Wk/Wv/Wp, q/k/v, p, atn) is bf16; every matmul
  accumulates in fp32 PSUM.  bf16 stationaries get FWL (fast weight load,
  ~53ns/128-col) so LDWEIGHTS hides under the 512-col moving pass.  Weights
  are host-swizzled so each on-chip slice is partition-contiguous in DRAM
  (4KB DMA packets instead of 256B).
- q^T/k^T ([d,t] layout) and v ([t,d] layout) stay resident in SBUF (12 MB)
  -- no DRAM spill roundtrips between projection and attention.
- Softmax in the S^T layout: scores^T [k,q] per 128-k-chunk; exp on the
  Scalar engine batched over two PSUM banks ([128,1024]) to amortize the
  ~352-cycle ACTIVATE overhead; causal masking is multiplicative (one
  [128,128] lower-triangle bf16 tile on DVE, only on the 4 diagonal 128x128
  sub-blocks per (head, q-group)); strictly-above-diagonal work is skipped,
  and the diagonal blocks are column-trimmed (packed variable-width scores /
  attn matmuls).
- Softmax denominator: p_sum accumulated on DVE in bf16 (2x mode), then ONE
  ones-matmul per (head, q-group) with a [128,128] ones stationary -- the
  matmul itself broadcasts the column-sums to all 128 partitions -- then a
  single-pass DVE reciprocal_approx_fast and a GPSIMD multiply into the
  resident bf16 attention tile.  No DRAM roundtrip.
- Output projection (Wp row-slice) is interleaved per q-group into the
  attention stream one q-group behind, so its PE work fills attention's
  ACT/DVE-bound stretches; y tiles DMA out on the (otherwise idle) Sync
  queue as they finish.
"""
import math
import os
from collections import deque

import ml_dtypes
import numpy as np

import concourse.bass as bass
import concourse.tile as tile
from concourse import bacc, mybir
from concourse.bass_utils import run_bass_kernel_spmd

f32 = mybir.dt.float32
bf16 = mybir.dt.bfloat16
AF = mybir.ActivationFunctionType
BF = ml_dtypes.bfloat16

N_CORES = 8
HD = 128                      # head dim

# results of the last run_bass_kernel_spmd call (for test harness profiling)
LAST_RESULT = None


def build_nc(T=2048, E=2048, D=1024, NOD=2048, TG=512, bias=False,
             num_devices=N_CORES):
    """Build + compile the per-core Bass program."""
    NH = D // HD              # heads per core
    EC = E // 128 + (1 if bias else 0)
    TC = T // 128             # 128-row tiles along T
    TGC = T // TG             # q-groups
    ODG = NOD // 512          # out-proj column groups
    VN = 512                  # v-projection moving width
    NVS = D // VN
    scale = 1.0 / math.sqrt(HD)

    nc = bacc.Bacc("TRN2", target_bir_lowering=False, debug=False,
                   num_devices=num_devices)

    xT_d = nc.dram_tensor("xT", [EC * 128, T], bf16, kind="ExternalInput")
    wq_d = nc.dram_tensor("wq", [D // 128, 128, EC * 128], bf16,
                          kind="ExternalInput")
    wk_d = nc.dram_tensor("wk", [D // 128, 128, EC * 128], bf16,
                          kind="ExternalInput")
    wv_d = nc.dram_tensor("wv", [NVS, 128, EC * VN], bf16,
                          kind="ExternalInput")
    wp_d = nc.dram_tensor("wp", [D, NOD], bf16, kind="ExternalInput")
    tri_d = nc.dram_tensor("tri", [128, 128], bf16, kind="ExternalInput")
    ones_d = nc.dram_tensor("ones", [128, 128], bf16, kind="ExternalInput")
    y_d = nc.dram_tensor("y", [T, NOD], f32, kind="ExternalOutput")

    with tile.TileContext(nc) as tc:
        with tc.tile_pool(name="persist", bufs=1) as persist:
            # q^T/k^T in [d, t] layout (head h = 128-row chunk h), v in
            # natural [t, d] layout ([t%128, tt*D + d]).
            qt_all = persist.tile([128, NH * T], bf16)
            kt_all = persist.tile([128, NH * T], bf16)
            v_all = persist.tile([128, TC * D], bf16)
            ones_sb = persist.tile([128, 128], bf16)
            tri_sb = persist.tile([128, 128], bf16)
            scr = persist.tile([1, 1], f32)
            nc.sync.dma_start(ones_sb[:], ones_d[:])
            nc.sync.dma_start(tri_sb[:], tri_d[:])

            # ---------------- phase A+B: q/k/v projections ----------------
            with (
                tc.tile_pool(name="xt", bufs=1) as xt_pool,
                tc.tile_pool(name="wcola", bufs=3) as wcol_pool,
                tc.tile_pool(name="wvp", bufs=1) as wv_pool,
                tc.tile_pool(name="ab_psum", bufs=8, space="PSUM") as ab_psum,
            ):
                xt_sb = xt_pool.tile([128, EC * T], bf16)

                def xt_e(e):
                    return xt_sb[:, e * T:(e + 1) * T]

                dsts = (qt_all, kt_all)
                wds = (wq_d, wk_d)
                wpairs = [(w_i, dc) for w_i in range(2) for dc in range(D // 128)]

                def load_wcol(w_i, dc):
                    wcol = wcol_pool.tile([128, EC * 128], bf16, tag="wcol",
                                          name=f"wcol_{w_i}_{dc}")
                    nc.sync.dma_start(wcol[:], wds[w_i][dc])
                    return wcol

                # xT split across both DMA queues (evens Sync, odds Scalar)
                # so chunk arrival outpaces the first wave pair's
                # e-consumption (~1.7us per chunk).
                wcol_q = [load_wcol(*wpairs[0])]
                nc.sync.dma_start(xt_sb[:, 0:T], xT_d[0:128, :])
                wcol_q.append(load_wcol(*wpairs[1]))
                for e in range(1, EC):
                    eng = nc.sync if e % 2 == 0 else nc.scalar
                    eng.dma_start(
                        xt_sb[:, e * T:(e + 1) * T],
                        xT_d[e * 128:(e + 1) * 128, :],
                    )
                # dummy exp (emitted after the xT triggers so it doesn't
                # block them): pulls the ~2.7us ACT table load off phase C's
                # critical path.
                nc.scalar.activation(scr[:], ones_sb[0:1, 0:1], AF.Exp,
                                     scale=1.0)
                wvgs = []
                for dg in range(NVS):
                    wvg = wv_pool.tile([128, EC * VN], bf16, tag=f"wvg{dg}",
                                       name=f"wvg_{dg}")
                    nc.sync.dma_start(wvg[:], wv_d[dg])
                    wvgs.append(wvg)

                # Q^T / K^T: one (weight, d-chunk) per wave of 4 PSUM banks,
                # e-major inside the wave; 8 banks = two waves in flight.
                # The first TWO waves are e-interleaved so each arriving xT
                # chunk feeds 8 matmuls, matching the chunk DMA arrival rate.
                ngrp = T // TG

                def qk_wave_tiles(w_i, dc):
                    return [ab_psum.tile([128, TG], f32, tag="abps",
                                         name=f"abps_{w_i}_{dc}_{tg}")
                            for tg in range(ngrp)]

                def qk_wave_mms(pss, wcol, e):
                    for tg in range(ngrp):
                        nc.tensor.matmul(
                            pss[tg][:],
                            wcol[:, e * 128:(e + 1) * 128],
                            xt_e(e)[:, tg * TG:(tg + 1) * TG],
                            start=(e == 0), stop=(e == EC - 1),
                        )

                def qk_wave_copies(pss, w_i, dc):
                    for tg in range(ngrp):
                        nc.scalar.copy(
                            dsts[w_i][:, dc * T + tg * TG:dc * T + (tg + 1) * TG],
                            pss[tg][:])

                wcol_q.append(load_wcol(*wpairs[2]))
                wcol_q.append(load_wcol(*wpairs[3]))
                pss0 = qk_wave_tiles(*wpairs[0])
                pss1 = qk_wave_tiles(*wpairs[1])
                wcol0, wcol1 = wcol_q.pop(0), wcol_q.pop(0)
                for e in range(EC):
                    qk_wave_mms(pss0, wcol0, e)
                    qk_wave_mms(pss1, wcol1, e)
                qk_wave_copies(pss0, *wpairs[0])
                qk_wave_copies(pss1, *wpairs[1])
                for wi in range(2, len(wpairs)):
                    w_i, dc = wpairs[wi]
                    wcol = wcol_q.pop(0)
                    if wi + 2 < len(wpairs):
                        wcol_q.append(load_wcol(*wpairs[wi + 2]))
                    pss = qk_wave_tiles(w_i, dc)
                    for e in range(EC):
                        qk_wave_mms(pss, wcol, e)
                    qk_wave_copies(pss, w_i, dc)

                # V in natural [t, d] layout at full 512 moving width.
                for tt in range(TC):
                    for dg in range(NVS):
                        ps = ab_psum.tile([128, VN], f32, tag="abps",
                                          name=f"vps_{tt}_{dg}")
                        for e in range(EC):
                            nc.tensor.matmul(
                                ps[:],
                                xt_e(e)[:, tt * 128:(tt + 1) * 128],
                                wvgs[dg][:, e * VN:(e + 1) * VN],
                                start=(e == 0), stop=(e == EC - 1),
                            )
                        nc.scalar.copy(
                            v_all[:, tt * D + dg * VN:tt * D + (dg + 1) * VN],
                            ps[:])

            # ---------------- phase C+D: attention + out-proj ----------------
            with (
                tc.tile_pool(name="cd", bufs=1) as cd_pool,
                tc.tile_pool(name="pt", bufs=3) as pt_pool,
                tc.tile_pool(name="sm", bufs=2) as sm_pool,
                tc.tile_pool(name="s_psum", bufs=2, space="PSUM") as s_psum,
                tc.tile_pool(name="a_psum", bufs=1, space="PSUM") as a_psum,
                tc.tile_pool(name="d_psum", bufs=1, space="PSUM") as d_psum,
                tc.tile_pool(name="y_psum", bufs=2, space="PSUM") as y_psum,
            ):
                atn_all = cd_pool.tile([128, NH * T], bf16)
                wp_sb = cd_pool.tile([128, NH * ODG * 512], bf16)
                nc.scalar.dma_start(
                    wp_sb.rearrange("p (hc og o) -> p hc og o", hc=NH, og=ODG),
                    wp_d.rearrange("(hc p) (og o) -> p hc og o", p=128, o=512),
                )
                dsum_t = d_psum.tile([128, TG], f32)

                def make_finalize(qg, h, p_sum, atn_u):
                    qbase = qg * TG
                    slot = qg * NH + h

                    def finalize():
                        # ones[128,128]-matmul broadcasts the column-sums of
                        # p_sum to every partition; single-pass DVE
                        # reciprocal; gpsimd multiply into atn_all.  Deferred
                        # past interleaved out-proj blocks so the DVE p_sum
                        # chain drains off the PE critical path.
                        nc.tensor.matmul(dsum_t[:], ones_sb[:], p_sum[:],
                                         start=True, stop=True)
                        recipB = sm_pool.tile([128, TG], f32, tag="rB",
                                              name=f"rB_{slot}")
                        nc.vector.reciprocal_approx_fast(out=recipB[:],
                                                         in_=dsum_t[:])
                        nc.gpsimd.tensor_mul(
                            atn_all[:, h * T + qbase:h * T + qbase + TG],
                            atn_u[:], recipB[:])

                    return finalize

                def emit_qg(qg, filler, fins):
                    """All NH heads of one q-group as a single flat pair
                    pipeline: consuming pair (h,p) is deferred one step past
                    producing pair (h,p+1), ACROSS head boundaries, so the
                    PE keeps lookahead scores work while each exp/mask chain
                    is in flight.  `filler(h)` runs early in head h's block
                    (after its second pair's scores) to interleave out-proj
                    blocks and deferred finalizers."""
                    qbase = qg * TG
                    npairs = 2 * (qg + 1)
                    nk = 4 * (qg + 1)
                    kc0 = qg * 4           # first diagonal k-chunk

                    def pair_desc(p):
                        # [(kc, soff, width, qoff)], exp width
                        if p == npairs - 2:
                            return [(kc0, 0, 512, 0),
                                    (kc0 + 1, 512, 384, 128)], 896
                        if p == npairs - 1:
                            return [(kc0 + 2, 0, 256, 256),
                                    (kc0 + 3, 256, 128, 384)], 384
                        return [(2 * p, 0, 512, 0),
                                (2 * p + 1, 512, 512, 0)], 1024

                    st = {}  # h -> {"pts", "atn", "psum"}

                    def emit_scores(h, p):
                        parts, expw = pair_desc(p)
                        s_pair = s_psum.tile([128, 2 * TG], f32, tag="sp",
                                             name=f"sp_{qg}_{h}_{p}")
                        # pair B packs both score blocks into one PSUM bank:
                        # exactly one start (bank pending-zero mark) and one
                        # stop per bank.
                        packed = p == npairs - 1
                        for pi, (kc, soff, w, qoff) in enumerate(parts):
                            nc.tensor.matmul(
                                s_pair[:, soff:soff + w],
                                kt_all[:, h * T + kc * 128:h * T + (kc + 1) * 128],
                                qt_all[:, h * T + qbase + qoff:h * T + qbase + 512],
                                start=(not packed or pi == 0),
                                stop=(not packed or pi == len(parts) - 1),
                            )
                        p_t = pt_pool.tile([128, 2 * TG], bf16, tag="pt",
                                           name=f"pt_{qg}_{h}_{p}")
                        st[h]["pts"][p] = p_t
                        nc.scalar.activation(p_t[:, 0:expw], s_pair[:, 0:expw],
                                             AF.Exp, scale=scale)
                        if p >= npairs - 2:
                            # multiplicative causal mask on the two 128-wide
                            # partial-triangle sections of this pair
                            for (kc, soff, w, qoff) in parts:
                                nc.vector.tensor_mul(
                                    p_t[:, soff:soff + 128],
                                    p_t[:, soff:soff + 128],
                                    tri_sb[:])

                    def emit_av(h, p):
                        # attn matmuls + DVE p-sum accumulation for pair p
                        # (after its exp/mask)
                        parts, _ = pair_desc(p)
                        g = st[h]
                        if p == 0:
                            # allocated at first use so pool-slot WAR edges
                            # cover every op already emitted on the previous
                            # occupant of the slot
                            g["atn"] = a_psum.tile([128, TG], f32, tag="atn",
                                                   name=f"atn_{qg}_{h}")
                            g["psum"] = pt_pool.tile([128, TG], bf16,
                                                     tag="psacc", bufs=2,
                                                     name=f"psacc_{qg}_{h}")
                        p_t = g["pts"][p]
                        for (kc, soff, w, qoff) in parts:
                            nc.tensor.matmul(
                                g["atn"][:, qoff:qoff + w],
                                v_all[:, kc * D + h * HD:kc * D + (h + 1) * HD],
                                p_t[:, soff:soff + w],
                                start=(kc == 0), stop=(kc == nk - 1),
                            )
                        for (kc, soff, w, qoff) in parts:
                            if kc == 0:
                                nc.vector.tensor_copy(g["psum"][:],
                                                      p_t[:, 0:TG])
                            else:
                                nc.vector.tensor_add(
                                    g["psum"][:, qoff:qoff + w],
                                    g["psum"][:, qoff:qoff + w],
                                    p_t[:, soff:soff + w])
                        if p == npairs - 1:
                            slot = qg * NH + h
                            atn_u = sm_pool.tile([128, TG], f32, tag="atnu",
                                                 name=f"atnu_{slot}")
                            nc.scalar.copy(atn_u[:], g["atn"][:])
                            fins.append(make_finalize(qg, h, g["psum"],
                                                      atn_u))
                            st.pop(h)

                    prev = None
                    for h in range(NH):
                        st[h] = {"pts": [None] * npairs}
                        for p in range(npairs):
                            emit_scores(h, p)
                            if prev is not None:
                                emit_av(*prev)
                            prev = (h, p)
                            if p == min(1, npairs - 1):
                                filler(h)
                    emit_av(*prev)

                def emit_dblock(tt, og):
                    ps = y_psum.tile([128, 512], f32, tag="yps",
                                     name=f"yps_{tt}_{og}")
                    for hc in range(NH):
                        nc.tensor.matmul(
                            ps[:],
                            atn_all[:, hc * T + tt * 128:hc * T + (tt + 1) * 128],
                            wp_sb[:, (hc * ODG + og) * 512:(hc * ODG + og + 1) * 512],
                            start=(hc == 0), stop=(hc == NH - 1),
                        )
                    yst = sm_pool.tile([128, 512], f32, tag="yst",
                                       name=f"yst_{tt}_{og}")
                    nc.vector.tensor_copy(yst[:], ps[:])
                    nc.sync.dma_start(
                        y_d[tt * 128:(tt + 1) * 128, og * 512:(og + 1) * 512],
                        yst[:])

                # Ascending q-groups; each group's ACT/DVE-bound stretches
                # are filled with the previous q-group's out-projection
                # blocks.  Finalizer ordering: mid-group, D-blocks go first
                # so the finalizer's ones-matmul lands after ~3.4us of PE
                # filler (DVE p_sum chain drained); at a q-group boundary
                # (h==0), ALL leftover finalizers of the previous q-group
                # flush BEFORE its first out-proj block, which reads the
                # atn_all slices those finalizers write.
                dq = deque()
                fins = deque()

                def filler(h):
                    if h == 0:
                        while fins:
                            fins.popleft()()
                    for _ in range(2):
                        if dq:
                            emit_dblock(*dq.popleft())
                    if h > 0 and fins:
                        fins.popleft()()

                for qg in range(TGC):
                    emit_qg(qg, filler, fins)
                    for tt in range(qg * 4, qg * 4 + 4):
                        for og in range(ODG):
                            dq.append((tt, og))
                while fins:
                    fins.popleft()()
                while dq:
                    emit_dblock(*dq.popleft())

    nc.compile()
    return nc


def _augment(mat, bias_row, pad_to):
    """Append [bias_row; zeros] below mat so it has pad_to rows."""
    extra = np.zeros((pad_to - mat.shape[0], mat.shape[1]), np.float32)
    extra[0] = bias_row
    return np.concatenate([mat, extra], axis=0)


def _swizzle_qk(w, EC):
    """[EC*128, D] -> [D//128, 128, EC*128]: per-wave slice partition-major
    so its DMA moves in 4KB packets."""
    D = w.shape[1]
    return np.ascontiguousarray(
        w.reshape(EC, 128, D // 128, 128).transpose(2, 1, 0, 3)
        .reshape(D // 128, 128, EC * 128).astype(BF))


def _swizzle_v(w, EC, VN=512):
    """[EC*128, D] -> [D//VN, 128, EC*VN] partition-major."""
    D = w.shape[1]
    return np.ascontiguousarray(
        w.reshape(EC, 128, D // VN, VN).transpose(2, 1, 0, 3)
        .reshape(D // VN, 128, EC * VN).astype(BF))


_NC_CACHE = {}


def _get_nc(bias):
    if bias not in _NC_CACHE:
        _NC_CACHE[bias] = build_nc(bias=bias)
    return _NC_CACHE[bias]


def kernel(x, Wq, bq, Wk, bk, Wv, bv, Wp, bp):
    global LAST_RESULT
    x = np.ascontiguousarray(np.asarray(x, np.float32))
    Wq, bq = np.asarray(Wq, np.float32), np.asarray(bq, np.float32)
    Wk, bk = np.asarray(Wk, np.float32), np.asarray(bk, np.float32)
    Wv, bv = np.asarray(Wv, np.float32), np.asarray(bv, np.float32)
    Wp, bp = np.asarray(Wp, np.float32), np.asarray(bp, np.float32)

    B, T, C = x.shape
    assert (B, T, C) == (4, 2048, 2048), (B, T, C)
    D = 1024  # head-group width: 8 heads per core
    bias = bool(np.any(bq) or np.any(bk) or np.any(bv))
    nc = _get_nc(bias)

    kk = np.arange(128)[:, None]
    qq = np.arange(128)[None, :]
    tri = (kk <= qq).astype(BF)
    ones = np.ones((128, 128), BF)
    Ep = C + 128 if bias else C

    in_maps = []
    for c in range(N_CORES):
        b, g = c // 2, c % 2
        xt = x[b].T
        wq_g = Wq[:, g * D:(g + 1) * D]
        wk_g = Wk[:, g * D:(g + 1) * D]
        wv_g = Wv[:, g * D:(g + 1) * D]
        if bias:
            xt = _augment(xt, np.ones(T, np.float32), Ep)
            wq_g = _augment(wq_g, bq[g * D:(g + 1) * D], Ep)
            wk_g = _augment(wk_g, bk[g * D:(g + 1) * D], Ep)
            wv_g = _augment(wv_g, bv[g * D:(g + 1) * D], Ep)
        EC = Ep // 128
        in_maps.append({
            "xT": np.ascontiguousarray(xt.astype(BF)),
            "wq": _swizzle_qk(wq_g, EC),
            "wk": _swizzle_qk(wk_g, EC),
            "wv": _swizzle_v(wv_g, EC),
            "wp": np.ascontiguousarray(Wp[g * D:(g + 1) * D, :].astype(BF)),
            "tri": tri,
            "ones": ones,
        })

    trace = bool(os.environ.get("MHA_TRACE"))
    res = run_bass_kernel_spmd(nc, in_maps, core_ids=list(range(N_CORES)),
                               trace=trace)
    LAST_RESULT = res

    out = np.empty((B, T, C), np.float32)
    for b in range(B):
        out[b] = res.results[2 * b]["y"] + res.results[2 * b + 1]["y"]
    out += bp[None, None, :]
    return out


# revision 27
# speedup vs baseline: 1.0034x; 1.0034x over previous
"""TRN2 Bass kernel for 16-head causal MHA (B=4, T=2048, C=2048), fp32 in/out.

Sharding: 8 cores = 4 batches x 2 head-groups (8 heads each).  Each core
computes q/k/v projections for its head group on its batch (tensor-parallel
column split of Wq/Wk/Wv), causal attention in the S^T layout, and a partial
output projection with the row slice of Wp.  The two head-group partials per
batch are summed on the host, plus the output bias.

Design (v6, bf16 datapath):
- All operand data (x^T, Wq/Wk/Wv/Wp, q/k/v, p, atn) is bf16; every matmul
  accumulates in fp32 PSUM.  bf16 stationaries get FWL (fast weight load,
  ~53ns/128-col) so LDWEIGHTS hides under the 512-col moving pass.  Weights
  are host-swizzled so each on-chip slice is partition-contiguous in DRAM
  (4KB DMA packets instead of 256B).
- q^T/k^T ([d,t] layout) and v ([t,d] layout) stay resident in SBUF (12 MB)
  -- no DRAM spill roundtrips between projection and attention.
- Softmax in the S^T layout: scores^T [k,q] per 128-k-chunk; exp on the
  Scalar engine batched over two PSUM banks ([128,1024]) to amortize the
  ~352-cycle ACTIVATE overhead; causal masking is multiplicative (one
  [128,128] lower-triangle bf16 tile on DVE, only on the 4 diagonal 128x128
  sub-blocks per (head, q-group)); strictly-above-diagonal work is skipped,
  and the diagonal blocks are column-trimmed (packed variable-width scores /
  attn matmuls).
- Softmax denominator: p_sum accumulated on DVE in bf16 (2x mode), then ONE
  ones-matmul per (head, q-group) with a [128,128] ones stationary -- the
  matmul itself broadcasts the column-sums to all 128 partitions -- then a
  single-pass DVE reciprocal_approx_fast and a GPSIMD multiply into the
  resident bf16 attention tile.  No DRAM roundtrip.
- Output projection (Wp row-slice) is interleaved per q-group into the
  attention stream one q-group behind, so its PE work fills attention's
  ACT/DVE-bound stretches; y tiles DMA out on the (otherwise idle) Sync
  queue as they finish.
"""
import math
import os
from collections import deque

import ml_dtypes
import numpy as np

import concourse.bass as bass
import concourse.tile as tile
from concourse import bacc, mybir
from concourse.bass_utils import run_bass_kernel_spmd

f32 = mybir.dt.float32
bf16 = mybir.dt.bfloat16
AF = mybir.ActivationFunctionType
BF = ml_dtypes.bfloat16

N_CORES = 8
HD = 128                      # head dim

# results of the last run_bass_kernel_spmd call (for test harness profiling)
LAST_RESULT = None


def build_nc(T=2048, E=2048, D=1024, NOD=2048, TG=512, bias=False,
             num_devices=N_CORES):
    """Build + compile the per-core Bass program."""
    NH = D // HD              # heads per core
    EC = E // 128 + (1 if bias else 0)
    TC = T // 128             # 128-row tiles along T
    TGC = T // TG             # q-groups
    ODG = NOD // 512          # out-proj column groups
    VN = 512                  # v-projection moving width
    NVS = D // VN
    scale = 1.0 / math.sqrt(HD)

    nc = bacc.Bacc("TRN2", target_bir_lowering=False, debug=False,
                   num_devices=num_devices)

    xT_d = nc.dram_tensor("xT", [EC * 128, T], bf16, kind="ExternalInput")
    wq_d = nc.dram_tensor("wq", [D // 128, 128, EC * 128], bf16,
                          kind="ExternalInput")
    wk_d = nc.dram_tensor("wk", [D // 128, 128, EC * 128], bf16,
                          kind="ExternalInput")
    wv_d = nc.dram_tensor("wv", [NVS, 128, EC * VN], bf16,
                          kind="ExternalInput")
    wp_d = nc.dram_tensor("wp", [D, NOD], bf16, kind="ExternalInput")
    tri_d = nc.dram_tensor("tri", [128, 128], bf16, kind="ExternalInput")
    ones_d = nc.dram_tensor("ones", [128, 128], bf16, kind="ExternalInput")
    y_d = nc.dram_tensor("y", [T, NOD], f32, kind="ExternalOutput")

    with tile.TileContext(nc) as tc:
        with tc.tile_pool(name="persist", bufs=1) as persist:
            # q^T/k^T in [d, t] layout (head h = 128-row chunk h), v in
            # natural [t, d] layout ([t%128, tt*D + d]).
            qt_all = persist.tile([128, NH * T], bf16)
            kt_all = persist.tile([128, NH * T], bf16)
            v_all = persist.tile([128, TC * D], bf16)
            ones_sb = persist.tile([128, 128], bf16)
            tri_sb = persist.tile([128, 128], bf16)
            scr = persist.tile([1, 1], f32)
            nc.sync.dma_start(ones_sb[:], ones_d[:])
            nc.sync.dma_start(tri_sb[:], tri_d[:])

            # ---------------- phase A+B: q/k/v projections ----------------
            with (
                tc.tile_pool(name="xt", bufs=1) as xt_pool,
                tc.tile_pool(name="wcola", bufs=3) as wcol_pool,
                tc.tile_pool(name="wvp", bufs=1) as wv_pool,
                tc.tile_pool(name="ab_psum", bufs=8, space="PSUM") as ab_psum,
            ):
                xt_sb = xt_pool.tile([128, EC * T], bf16)

                def xt_e(e):
                    return xt_sb[:, e * T:(e + 1) * T]

                dsts = (qt_all, kt_all)
                wds = (wq_d, wk_d)
                wpairs = [(w_i, dc) for w_i in range(2) for dc in range(D // 128)]

                def load_wcol(w_i, dc):
                    wcol = wcol_pool.tile([128, EC * 128], bf16, tag="wcol",
                                          name=f"wcol_{w_i}_{dc}")
                    nc.sync.dma_start(wcol[:], wds[w_i][dc])
                    return wcol

                # xT split across both DMA queues (evens Sync, odds Scalar)
                # so chunk arrival outpaces the first wave pair's
                # e-consumption (~1.7us per chunk).
                wcol_q = [load_wcol(*wpairs[0])]
                nc.sync.dma_start(xt_sb[:, 0:T], xT_d[0:128, :])
                wcol_q.append(load_wcol(*wpairs[1]))
                for e in range(1, EC):
                    eng = nc.sync if e % 2 == 0 else nc.scalar
                    eng.dma_start(
                        xt_sb[:, e * T:(e + 1) * T],
                        xT_d[e * 128:(e + 1) * 128, :],
                    )
                # dummy exp (emitted after the xT triggers so it doesn't
                # block them): pulls the ~2.7us ACT table load off phase C's
                # critical path.
                nc.scalar.activation(scr[:], ones_sb[0:1, 0:1], AF.Exp,
                                     scale=1.0)
                wvgs = []
                for dg in range(NVS):
                    wvg = wv_pool.tile([128, EC * VN], bf16, tag=f"wvg{dg}",
                                       name=f"wvg_{dg}")
                    nc.sync.dma_start(wvg[:], wv_d[dg])
                    wvgs.append(wvg)

                # Q^T / K^T: one (weight, d-chunk) per wave of 4 PSUM banks,
                # e-major inside the wave; 8 banks = two waves in flight.
                # The first TWO waves are e-interleaved so each arriving xT
                # chunk feeds 8 matmuls, matching the chunk DMA arrival rate.
                ngrp = T // TG

                def qk_wave_tiles(w_i, dc):
                    return [ab_psum.tile([128, TG], f32, tag="abps",
                                         name=f"abps_{w_i}_{dc}_{tg}")
                            for tg in range(ngrp)]

                def qk_wave_mms(pss, wcol, e):
                    for tg in range(ngrp):
                        nc.tensor.matmul(
                            pss[tg][:],
                            wcol[:, e * 128:(e + 1) * 128],
                            xt_e(e)[:, tg * TG:(tg + 1) * TG],
                            start=(e == 0), stop=(e == EC - 1),
                        )

                def qk_wave_copies(pss, w_i, dc):
                    for tg in range(ngrp):
                        nc.scalar.copy(
                            dsts[w_i][:, dc * T + tg * TG:dc * T + (tg + 1) * TG],
                            pss[tg][:])

                wcol_q.append(load_wcol(*wpairs[2]))
                wcol_q.append(load_wcol(*wpairs[3]))
                pss0 = qk_wave_tiles(*wpairs[0])
                pss1 = qk_wave_tiles(*wpairs[1])
                wcol0, wcol1 = wcol_q.pop(0), wcol_q.pop(0)
                for e in range(EC):
                    qk_wave_mms(pss0, wcol0, e)
                    qk_wave_mms(pss1, wcol1, e)
                qk_wave_copies(pss0, *wpairs[0])
                qk_wave_copies(pss1, *wpairs[1])
                for wi in range(2, len(wpairs)):
                    w_i, dc = wpairs[wi]
                    wcol = wcol_q.pop(0)
                    if wi + 2 < len(wpairs):
                        wcol_q.append(load_wcol(*wpairs[wi + 2]))
                    pss = qk_wave_tiles(w_i, dc)
                    for e in range(EC):
                        qk_wave_mms(pss, wcol, e)
                    qk_wave_copies(pss, w_i, dc)

                # V in natural [t, d] layout at full 512 moving width.
                for tt in range(TC):
                    for dg in range(NVS):
                        ps = ab_psum.tile([128, VN], f32, tag="abps",
                                          name=f"vps_{tt}_{dg}")
                        for e in range(EC):
                            nc.tensor.matmul(
                                ps[:],
                                xt_e(e)[:, tt * 128:(tt + 1) * 128],
                                wvgs[dg][:, e * VN:(e + 1) * VN],
                                start=(e == 0), stop=(e == EC - 1),
                            )
                        nc.scalar.copy(
                            v_all[:, tt * D + dg * VN:tt * D + (dg + 1) * VN],
                            ps[:])

            # ---------------- phase C+D: attention + out-proj ----------------
            with (
                tc.tile_pool(name="cd", bufs=1) as cd_pool,
                tc.tile_pool(name="pt", bufs=3) as pt_pool,
                tc.tile_pool(name="sm", bufs=2) as sm_pool,
                tc.tile_pool(name="s_psum", bufs=2, space="PSUM") as s_psum,
                tc.tile_pool(name="a_psum", bufs=1, space="PSUM") as a_psum,
                tc.tile_pool(name="d_psum", bufs=1, space="PSUM") as d_psum,
                tc.tile_pool(name="y_psum", bufs=2, space="PSUM") as y_psum,
            ):
                atn_all = cd_pool.tile([128, NH * T], bf16)
                wp_sb = cd_pool.tile([128, NH * ODG * 512], bf16)
                nc.scalar.dma_start(
                    wp_sb.rearrange("p (hc og o) -> p hc og o", hc=NH, og=ODG),
                    wp_d.rearrange("(hc p) (og o) -> p hc og o", p=128, o=512),
                )
                dsum_t = d_psum.tile([128, TG], f32)

                def emit_cgroup(qg, h):
                    qbase = qg * TG
                    npairs = 2 * (qg + 1)
                    nk = 4 * (qg + 1)
                    kc0 = qg * 4           # first diagonal k-chunk

                    def pair_desc(p):
                        # [(kc, soff, width, qoff)], exp width
                        if p == npairs - 2:
                            return [(kc0, 0, 512, 0),
                                    (kc0 + 1, 512, 384, 128)], 896
                        if p == npairs - 1:
                            return [(kc0 + 2, 0, 256, 256),
                                    (kc0 + 3, 256, 128, 384)], 384
                        return [(2 * p, 0, 512, 0),
                                (2 * p + 1, 512, 512, 0)], 1024

                    pts = [None] * npairs
                    p_sum = pt_pool.tile([128, TG], bf16, tag="psacc",
                                         bufs=2, name=f"psacc_{qg}_{h}")

                    def emit_av(p):
                        # attn matmuls + DVE p-sum accumulation for pair p
                        # (after its exp/mask)
                        parts, _ = pair_desc(p)
                        p_t = pts[p]
                        for (kc, soff, w, qoff) in parts:
                            nc.tensor.matmul(
                                atn_ps[:, qoff:qoff + w],
                                v_all[:, kc * D + h * HD:kc * D + (h + 1) * HD],
                                p_t[:, soff:soff + w],
                                start=(kc == 0), stop=(kc == nk - 1),
                            )
                        for (kc, soff, w, qoff) in parts:
                            if kc == 0:
                                nc.vector.tensor_copy(p_sum[:], p_t[:, 0:TG])
                            else:
                                nc.vector.tensor_add(
                                    p_sum[:, qoff:qoff + w],
                                    p_sum[:, qoff:qoff + w],
                                    p_t[:, soff:soff + w])

                    atn_ps = a_psum.tile([128, TG], f32, tag="atn",
                                         name=f"atn_{qg}_{h}")
                    for p in range(npairs):
                        parts, expw = pair_desc(p)
                        s_pair = s_psum.tile([128, 2 * TG], f32, tag="sp",
                                             name=f"sp_{qg}_{h}_{p}")
                        # pair B packs both score blocks into one PSUM bank:
                        # exactly one start (bank pending-zero mark) and one
                        # stop per bank.
                        packed = p == npairs - 1
                        for pi, (kc, soff, w, qoff) in enumerate(parts):
                            nc.tensor.matmul(
                                s_pair[:, soff:soff + w],
                                kt_all[:, h * T + kc * 128:h * T + (kc + 1) * 128],
                                qt_all[:, h * T + qbase + qoff:h * T + qbase + 512],
                                start=(not packed or pi == 0),
                                stop=(not packed or pi == len(parts) - 1),
                            )
                        p_t = pt_pool.tile([128, 2 * TG], bf16, tag="pt",
                                           name=f"pt_{qg}_{h}_{p}")
                        pts[p] = p_t
                        nc.scalar.activation(p_t[:, 0:expw], s_pair[:, 0:expw],
                                             AF.Exp, scale=scale)
                        if p >= npairs - 2:
                            # multiplicative causal mask on the two 128-wide
                            # partial-triangle sections of this pair
                            for (kc, soff, w, qoff) in parts:
                                nc.vector.tensor_mul(
                                    p_t[:, soff:soff + 128],
                                    p_t[:, soff:soff + 128],
                                    tri_sb[:])
                        if p > 0:
                            emit_av(p - 1)
                    emit_av(npairs - 1)
                    slot = qg * NH + h
                    atn_u = sm_pool.tile([128, TG], f32, tag="atnu",
                                         name=f"atnu_{slot}")
                    nc.scalar.copy(atn_u[:], atn_ps[:])

                    def finalize():
                        # ones[128,128]-matmul broadcasts the column-sums of
                        # p_sum to every partition; single-pass DVE
                        # reciprocal; gpsimd multiply into atn_all.  Deferred
                        # past the interleaved out-proj blocks so the DVE
                        # p_sum chain drains off the PE critical path.
                        nc.tensor.matmul(dsum_t[:], ones_sb[:], p_sum[:],
                                         start=True, stop=True)
                        recipB = sm_pool.tile([128, TG], f32, tag="rB",
                                              name=f"rB_{slot}")
                        nc.vector.reciprocal_approx_fast(out=recipB[:],
                                                         in_=dsum_t[:])
                        nc.gpsimd.tensor_mul(
                            atn_all[:, h * T + qbase:h * T + qbase + TG],
                            atn_u[:], recipB[:])

                    return finalize

                def emit_dblock(tt, og):
                    ps = y_psum.tile([128, 512], f32, tag="yps",
                                     name=f"yps_{tt}_{og}")
                    for hc in range(NH):
                        nc.tensor.matmul(
                            ps[:],
                            atn_all[:, hc * T + tt * 128:hc * T + (tt + 1) * 128],
                            wp_sb[:, (hc * ODG + og) * 512:(hc * ODG + og + 1) * 512],
                            start=(hc == 0), stop=(hc == NH - 1),
                        )
                    yst = sm_pool.tile([128, 512], f32, tag="yst",
                                       name=f"yst_{tt}_{og}")
                    nc.vector.tensor_copy(yst[:], ps[:])
                    nc.sync.dma_start(
                        y_d[tt * 128:(tt + 1) * 128, og * 512:(og + 1) * 512],
                        yst[:])

                # Ascending q-groups; each group's ACT/DVE-bound stretches
                # are filled with the previous q-group's out-projection
                # blocks; normalization finalizers deferred past them.
                dq = deque()
                for qg in range(TGC):
                    for h in range(NH):
                        fin = emit_cgroup(qg, h)
                        for _ in range(2):
                            if dq:
                                emit_dblock(*dq.popleft())
                        fin()
                    for tt in range(qg * 4, qg * 4 + 4):
                        for og in range(ODG):
                            dq.append((tt, og))
                while dq:
                    emit_dblock(*dq.popleft())

    nc.compile()
    return nc


def _augment(mat, bias_row, pad_to):
    """Append [bias_row; zeros] below mat so it has pad_to rows."""
    extra = np.zeros((pad_to - mat.shape[0], mat.shape[1]), np.float32)
    extra[0] = bias_row
    return np.concatenate([mat, extra], axis=0)


def _swizzle_qk(w, EC):
    """[EC*128, D] -> [D//128, 128, EC*128]: per-wave slice partition-major
    so its DMA moves in 4KB packets."""
    D = w.shape[1]
    return np.ascontiguousarray(
        w.reshape(EC, 128, D // 128, 128).transpose(2, 1, 0, 3)
        .reshape(D // 128, 128, EC * 128).astype(BF))


def _swizzle_v(w, EC, VN=512):
    """[EC*128, D] -> [D//VN, 128, EC*VN] partition-major."""
    D = w.shape[1]
    return np.ascontiguousarray(
        w.reshape(EC, 128, D // VN, VN).transpose(2, 1, 0, 3)
        .reshape(D // VN, 128, EC * VN).astype(BF))


_NC_CACHE = {}


def _get_nc(bias):
    if bias not in _NC_CACHE:
        _NC_CACHE[bias] = build_nc(bias=bias)
    return _NC_CACHE[bias]


def kernel(x, Wq, bq, Wk, bk, Wv, bv, Wp, bp):
    global LAST_RESULT
    x = np.ascontiguousarray(np.asarray(x, np.float32))
    Wq, bq = np.asarray(Wq, np.float32), np.asarray(bq, np.float32)
    Wk, bk = np.asarray(Wk, np.float32), np.asarray(bk, np.float32)
    Wv, bv = np.asarray(Wv, np.float32), np.asarray(bv, np.float32)
    Wp, bp = np.asarray(Wp, np.float32), np.asarray(bp, np.float32)

    B, T, C = x.shape
    assert (B, T, C) == (4, 2048, 2048), (B, T, C)
    D = 1024  # head-group width: 8 heads per core
    bias = bool(np.any(bq) or np.any(bk) or np.any(bv))
    nc = _get_nc(bias)

    kk = np.arange(128)[:, None]
    qq = np.arange(128)[None, :]
    tri = (kk <= qq).astype(BF)
    ones = np.ones((128, 128), BF)
    Ep = C + 128 if bias else C

    in_maps = []
    for c in range(N_CORES):
        b, g = c // 2, c % 2
        xt = x[b].T
        wq_g = Wq[:, g * D:(g + 1) * D]
        wk_g = Wk[:, g * D:(g + 1) * D]
        wv_g = Wv[:, g * D:(g + 1) * D]
        if bias:
            xt = _augment(xt, np.ones(T, np.float32), Ep)
            wq_g = _augment(wq_g, bq[g * D:(g + 1) * D], Ep)
            wk_g = _augment(wk_g, bk[g * D:(g + 1) * D], Ep)
            wv_g = _augment(wv_g, bv[g * D:(g + 1) * D], Ep)
        EC = Ep // 128
        in_maps.append({
            "xT": np.ascontiguousarray(xt.astype(BF)),
            "wq": _swizzle_qk(wq_g, EC),
            "wk": _swizzle_qk(wk_g, EC),
            "wv": _swizzle_v(wv_g, EC),
            "wp": np.ascontiguousarray(Wp[g * D:(g + 1) * D, :].astype(BF)),
            "tri": tri,
            "ones": ones,
        })

    trace = bool(os.environ.get("MHA_TRACE"))
    res = run_bass_kernel_spmd(nc, in_maps, core_ids=list(range(N_CORES)),
                               trace=trace)
    LAST_RESULT = res

    out = np.empty((B, T, C), np.float32)
    for b in range(B):
        out[b] = res.results[2 * b]["y"] + res.results[2 * b + 1]["y"]
    out += bp[None, None, :]
    return out


# revision 29
# speedup vs baseline: 1.0072x; 1.0037x over previous
"""TRN2 Bass kernel for 16-head causal MHA (B=4, T=2048, C=2048), fp32 in/out.

Sharding: 8 cores = 4 batches x 2 head-groups (8 heads each).  Each core
computes q/k/v projections for its head group on its batch (tensor-parallel
column split of Wq/Wk/Wv), causal attention in the S^T layout, and a partial
output projection with the row slice of Wp.  The two head-group partials per
batch are summed on the host, plus the output bias.

Design (v6, bf16 datapath):
- All operand data (x^T, Wq/Wk/Wv/Wp, q/k/v, p, atn) is bf16; every matmul
  accumulates in fp32 PSUM.  bf16 stationaries get FWL (fast weight load,
  ~53ns/128-col) so LDWEIGHTS hides under the 512-col moving pass.  Weights
  are host-swizzled so each on-chip slice is partition-contiguous in DRAM
  (4KB DMA packets instead of 256B).
- q^T/k^T ([d,t] layout) and v ([t,d] layout) stay resident in SBUF (12 MB)
  -- no DRAM spill roundtrips between projection and attention.
- Softmax in the S^T layout: scores^T [k,q] per 128-k-chunk; exp on the
  Scalar engine batched over two PSUM banks ([128,1024]) to amortize the
  ~352-cycle ACTIVATE overhead; causal masking is multiplicative (one
  [128,128] lower-triangle bf16 tile on DVE, only on the 4 diagonal 128x128
  sub-blocks per (head, q-group)); strictly-above-diagonal work is skipped,
  and the diagonal blocks are column-trimmed (packed variable-width scores /
  attn matmuls).
- Softmax denominator: p_sum accumulated on DVE in bf16 (2x mode), then ONE
  ones-matmul per (head, q-group) with a [128,128] ones stationary -- the
  matmul itself broadcasts the column-sums to all 128 partitions -- then a
  single-pass DVE reciprocal_approx_fast and a GPSIMD multiply into the
  resident bf16 attention tile.  No DRAM roundtrip.
- Output projection (Wp row-slice) is interleaved per q-group into the
  attention stream one q-group behind, so its PE work fills attention's
  ACT/DVE-bound stretches; y tiles DMA out on the (otherwise idle) Sync
  queue as they finish.
"""
import math
import os
from collections import deque

import ml_dtypes
import numpy as np

import concourse.bass as bass
import concourse.tile as tile
from concourse import bacc, mybir
from concourse.bass_utils import run_bass_kernel_spmd

f32 = mybir.dt.float32
bf16 = mybir.dt.bfloat16
AF = mybir.ActivationFunctionType
BF = ml_dtypes.bfloat16

N_CORES = 8
HD = 128                      # head dim

# results of the last run_bass_kernel_spmd call (for test harness profiling)
LAST_RESULT = None


def build_nc(T=2048, E=2048, D=1024, NOD=2048, TG=512, bias=False,
             num_devices=N_CORES):
    """Build + compile the per-core Bass program."""
    NH = D // HD              # heads per core
    EC = E // 128 + (1 if bias else 0)
    TC = T // 128             # 128-row tiles along T
    TGC = T // TG             # q-groups
    ODG = NOD // 512          # out-proj column groups
    VN = 512                  # v-projection moving width
    NVS = D // VN
    scale = 1.0 / math.sqrt(HD)

    nc = bacc.Bacc("TRN2", target_bir_lowering=False, debug=False,
                   num_devices=num_devices)

    xT_d = nc.dram_tensor("xT", [EC * 128, T], bf16, kind="ExternalInput")
    wq_d = nc.dram_tensor("wq", [D // 128, 128, EC * 128], bf16,
                          kind="ExternalInput")
    wk_d = nc.dram_tensor("wk", [D // 128, 128, EC * 128], bf16,
                          kind="ExternalInput")
    wv_d = nc.dram_tensor("wv", [NVS, 128, EC * VN], bf16,
                          kind="ExternalInput")
    wp_d = nc.dram_tensor("wp", [D, NOD], bf16, kind="ExternalInput")
    tri_d = nc.dram_tensor("tri", [128, 128], bf16, kind="ExternalInput")
    ones_d = nc.dram_tensor("ones", [128, 128], bf16, kind="ExternalInput")
    y_d = nc.dram_tensor("y", [T, NOD], f32, kind="ExternalOutput")

    with tile.TileContext(nc) as tc:
        with tc.tile_pool(name="persist", bufs=1) as persist:
            # q^T/k^T in [d, t] layout (head h = 128-row chunk h), v in
            # natural [t, d] layout ([t%128, tt*D + d]).
            qt_all = persist.tile([128, NH * T], bf16)
            kt_all = persist.tile([128, NH * T], bf16)
            v_all = persist.tile([128, TC * D], bf16)
            ones_sb = persist.tile([128, 128], bf16)
            tri_sb = persist.tile([128, 128], bf16)
            scr = persist.tile([1, 1], f32)
            nc.sync.dma_start(ones_sb[:], ones_d[:])
            nc.sync.dma_start(tri_sb[:], tri_d[:])

            # ---------------- phase A+B: q/k/v projections ----------------
            with (
                tc.tile_pool(name="xt", bufs=1) as xt_pool,
                tc.tile_pool(name="wcola", bufs=3) as wcol_pool,
                tc.tile_pool(name="wvp", bufs=1) as wv_pool,
                tc.tile_pool(name="ab_psum", bufs=8, space="PSUM") as ab_psum,
            ):
                xt_sb = xt_pool.tile([128, EC * T], bf16)

                def xt_e(e):
                    return xt_sb[:, e * T:(e + 1) * T]

                dsts = (qt_all, kt_all)
                wds = (wq_d, wk_d)
                wpairs = [(w_i, dc) for w_i in range(2) for dc in range(D // 128)]

                def load_wcol(w_i, dc):
                    wcol = wcol_pool.tile([128, EC * 128], bf16, tag="wcol",
                                          name=f"wcol_{w_i}_{dc}")
                    nc.sync.dma_start(wcol[:], wds[w_i][dc])
                    return wcol

                # xT split across both DMA queues (evens Sync, odds Scalar)
                # so chunk arrival outpaces the first wave pair's
                # e-consumption (~1.7us per chunk).
                wcol_q = [load_wcol(*wpairs[0])]
                nc.sync.dma_start(xt_sb[:, 0:T], xT_d[0:128, :])
                wcol_q.append(load_wcol(*wpairs[1]))
                for e in range(1, EC):
                    eng = nc.sync if e % 2 == 0 else nc.scalar
                    eng.dma_start(
                        xt_sb[:, e * T:(e + 1) * T],
                        xT_d[e * 128:(e + 1) * 128, :],
                    )
                # dummy exp (emitted after the xT triggers so it doesn't
                # block them): pulls the ~2.7us ACT table load off phase C's
                # critical path.
                nc.scalar.activation(scr[:], ones_sb[0:1, 0:1], AF.Exp,
                                     scale=1.0)
                wvgs = []
                for dg in range(NVS):
                    wvg = wv_pool.tile([128, EC * VN], bf16, tag=f"wvg{dg}",
                                       name=f"wvg_{dg}")
                    nc.sync.dma_start(wvg[:], wv_d[dg])
                    wvgs.append(wvg)

                # Q^T / K^T: one (weight, d-chunk) per wave of 4 PSUM banks,
                # e-major inside the wave; 8 banks = two waves in flight.
                # The first TWO waves are e-interleaved so each arriving xT
                # chunk feeds 8 matmuls, matching the chunk DMA arrival rate.
                ngrp = T // TG

                def qk_wave_tiles(w_i, dc):
                    return [ab_psum.tile([128, TG], f32, tag="abps",
                                         name=f"abps_{w_i}_{dc}_{tg}")
                            for tg in range(ngrp)]

                def qk_wave_mms(pss, wcol, e):
                    for tg in range(ngrp):
                        nc.tensor.matmul(
                            pss[tg][:],
                            wcol[:, e * 128:(e + 1) * 128],
                            xt_e(e)[:, tg * TG:(tg + 1) * TG],
                            start=(e == 0), stop=(e == EC - 1),
                        )

                def qk_wave_copies(pss, w_i, dc):
                    for tg in range(ngrp):
                        nc.scalar.copy(
                            dsts[w_i][:, dc * T + tg * TG:dc * T + (tg + 1) * TG],
                            pss[tg][:])

                wcol_q.append(load_wcol(*wpairs[2]))
                wcol_q.append(load_wcol(*wpairs[3]))
                pss0 = qk_wave_tiles(*wpairs[0])
                pss1 = qk_wave_tiles(*wpairs[1])
                wcol0, wcol1 = wcol_q.pop(0), wcol_q.pop(0)
                for e in range(EC):
                    qk_wave_mms(pss0, wcol0, e)
                    qk_wave_mms(pss1, wcol1, e)
                qk_wave_copies(pss0, *wpairs[0])
                qk_wave_copies(pss1, *wpairs[1])
                for wi in range(2, len(wpairs)):
                    w_i, dc = wpairs[wi]
                    wcol = wcol_q.pop(0)
                    if wi + 2 < len(wpairs):
                        wcol_q.append(load_wcol(*wpairs[wi + 2]))
                    pss = qk_wave_tiles(w_i, dc)
                    for e in range(EC):
                        qk_wave_mms(pss, wcol, e)
                    qk_wave_copies(pss, w_i, dc)

                # V in natural [t, d] layout at full 512 moving width.
                # Copies alternate ACT/DVE so the tail backlog at the
                # PSUM-pool phase handoff drains twice as fast.
                for tt in range(TC):
                    for dg in range(NVS):
                        ps = ab_psum.tile([128, VN], f32, tag="abps",
                                          name=f"vps_{tt}_{dg}")
                        for e in range(EC):
                            nc.tensor.matmul(
                                ps[:],
                                xt_e(e)[:, tt * 128:(tt + 1) * 128],
                                wvgs[dg][:, e * VN:(e + 1) * VN],
                                start=(e == 0), stop=(e == EC - 1),
                            )
                        dst = v_all[:, tt * D + dg * VN:tt * D + (dg + 1) * VN]
                        if (tt * NVS + dg) % 2:
                            nc.vector.tensor_copy(dst, ps[:])
                        else:
                            nc.scalar.copy(dst, ps[:])

            # ---------------- phase C+D: attention + out-proj ----------------
            with (
                tc.tile_pool(name="cd", bufs=1) as cd_pool,
                tc.tile_pool(name="pt", bufs=3) as pt_pool,
                tc.tile_pool(name="sm", bufs=2) as sm_pool,
                tc.tile_pool(name="s_psum", bufs=2, space="PSUM") as s_psum,
                tc.tile_pool(name="a_psum", bufs=1, space="PSUM") as a_psum,
                tc.tile_pool(name="d_psum", bufs=1, space="PSUM") as d_psum,
                tc.tile_pool(name="y_psum", bufs=2, space="PSUM") as y_psum,
            ):
                atn_all = cd_pool.tile([128, NH * T], bf16)
                wp_sb = cd_pool.tile([128, NH * ODG * 512], bf16)
                nc.scalar.dma_start(
                    wp_sb.rearrange("p (hc og o) -> p hc og o", hc=NH, og=ODG),
                    wp_d.rearrange("(hc p) (og o) -> p hc og o", p=128, o=512),
                )
                dsum_t = d_psum.tile([128, TG], f32)

                def emit_cgroup(qg, h):
                    qbase = qg * TG
                    npairs = 2 * (qg + 1)
                    nk = 4 * (qg + 1)
                    kc0 = qg * 4           # first diagonal k-chunk

                    def pair_desc(p):
                        # [(kc, soff, width, qoff)], exp width
                        if p == npairs - 2:
                            return [(kc0, 0, 512, 0),
                                    (kc0 + 1, 512, 384, 128)], 896
                        if p == npairs - 1:
                            return [(kc0 + 2, 0, 256, 256),
                                    (kc0 + 3, 256, 128, 384)], 384
                        return [(2 * p, 0, 512, 0),
                                (2 * p + 1, 512, 512, 0)], 1024

                    pts = [None] * npairs
                    p_sum = pt_pool.tile([128, TG], bf16, tag="psacc",
                                         bufs=2, name=f"psacc_{qg}_{h}")

                    def emit_av(p):
                        # attn matmuls + DVE p-sum accumulation for pair p
                        # (after its exp/mask)
                        parts, _ = pair_desc(p)
                        p_t = pts[p]
                        for (kc, soff, w, qoff) in parts:
                            nc.tensor.matmul(
                                atn_ps[:, qoff:qoff + w],
                                v_all[:, kc * D + h * HD:kc * D + (h + 1) * HD],
                                p_t[:, soff:soff + w],
                                start=(kc == 0), stop=(kc == nk - 1),
                            )
                        for (kc, soff, w, qoff) in parts:
                            if kc == 0:
                                nc.vector.tensor_copy(p_sum[:], p_t[:, 0:TG])
                            else:
                                nc.vector.tensor_add(
                                    p_sum[:, qoff:qoff + w],
                                    p_sum[:, qoff:qoff + w],
                                    p_t[:, soff:soff + w])

                    atn_ps = a_psum.tile([128, TG], f32, tag="atn",
                                         name=f"atn_{qg}_{h}")
                    for p in range(npairs):
                        parts, expw = pair_desc(p)
                        s_pair = s_psum.tile([128, 2 * TG], f32, tag="sp",
                                             name=f"sp_{qg}_{h}_{p}")
                        # pair B packs both score blocks into one PSUM bank:
                        # exactly one start (bank pending-zero mark) and one
                        # stop per bank.
                        packed = p == npairs - 1
                        for pi, (kc, soff, w, qoff) in enumerate(parts):
                            nc.tensor.matmul(
                                s_pair[:, soff:soff + w],
                                kt_all[:, h * T + kc * 128:h * T + (kc + 1) * 128],
                                qt_all[:, h * T + qbase + qoff:h * T + qbase + 512],
                                start=(not packed or pi == 0),
                                stop=(not packed or pi == len(parts) - 1),
                            )
                        p_t = pt_pool.tile([128, 2 * TG], bf16, tag="pt",
                                           name=f"pt_{qg}_{h}_{p}")
                        pts[p] = p_t
                        nc.scalar.activation(p_t[:, 0:expw], s_pair[:, 0:expw],
                                             AF.Exp, scale=scale)
                        if p >= npairs - 2:
                            # multiplicative causal mask on the two 128-wide
                            # partial-triangle sections of this pair
                            for (kc, soff, w, qoff) in parts:
                                nc.vector.tensor_mul(
                                    p_t[:, soff:soff + 128],
                                    p_t[:, soff:soff + 128],
                                    tri_sb[:])
                        if p > 0:
                            emit_av(p - 1)
                    emit_av(npairs - 1)
                    slot = qg * NH + h
                    atn_u = sm_pool.tile([128, TG], f32, tag="atnu",
                                         name=f"atnu_{slot}")
                    nc.scalar.copy(atn_u[:], atn_ps[:])

                    def finalize():
                        # ones[128,128]-matmul broadcasts the column-sums of
                        # p_sum to every partition; single-pass DVE
                        # reciprocal; gpsimd multiply into atn_all.  Deferred
                        # past the interleaved out-proj blocks so the DVE
                        # p_sum chain drains off the PE critical path.
                        nc.tensor.matmul(dsum_t[:], ones_sb[:], p_sum[:],
                                         start=True, stop=True)
                        recipB = sm_pool.tile([128, TG], f32, tag="rB",
                                              name=f"rB_{slot}")
                        nc.vector.reciprocal_approx_fast(out=recipB[:],
                                                         in_=dsum_t[:])
                        nc.gpsimd.tensor_mul(
                            atn_all[:, h * T + qbase:h * T + qbase + TG],
                            atn_u[:], recipB[:])

                    return finalize

                def emit_dblock(tt, og):
                    ps = y_psum.tile([128, 512], f32, tag="yps",
                                     name=f"yps_{tt}_{og}")
                    for hc in range(NH):
                        nc.tensor.matmul(
                            ps[:],
                            atn_all[:, hc * T + tt * 128:hc * T + (tt + 1) * 128],
                            wp_sb[:, (hc * ODG + og) * 512:(hc * ODG + og + 1) * 512],
                            start=(hc == 0), stop=(hc == NH - 1),
                        )
                    yst = sm_pool.tile([128, 512], f32, tag="yst",
                                       name=f"yst_{tt}_{og}")
                    nc.vector.tensor_copy(yst[:], ps[:])
                    nc.sync.dma_start(
                        y_d[tt * 128:(tt + 1) * 128, og * 512:(og + 1) * 512],
                        yst[:])

                # Ascending q-groups; each group's ACT/DVE-bound stretches
                # are filled with the previous q-group's out-projection
                # blocks; normalization finalizers deferred past them.
                warm_n = [0]

                def emit_warm(n):
                    # consumer-less matmuls into a y-pool tile: PE filler for
                    # the (filler-less) qg=0 stretch so exp-latency stalls
                    # don't cross HAM's idle window and re-throttle the PE.
                    # Later real D-blocks overwrite the tile (start=True).
                    warm_n[0] += 1
                    ps = y_psum.tile([128, 512], f32, tag="yps",
                                     name=f"warm_{warm_n[0]}")
                    for i in range(n):
                        nc.tensor.matmul(ps[:], kt_all[:, 0:128],
                                         qt_all[:, 0:512],
                                         start=(i == 0), stop=(i == n - 1))

                dq = deque()
                for qg in range(TGC):
                    for h in range(NH):
                        fin = emit_cgroup(qg, h)
                        if dq:
                            for _ in range(2):
                                if dq:
                                    emit_dblock(*dq.popleft())
                        else:
                            emit_warm(4)
                        fin()
                    for tt in range(qg * 4, qg * 4 + 4):
                        for og in range(ODG):
                            dq.append((tt, og))
                while dq:
                    emit_dblock(*dq.popleft())

    nc.compile()
    return nc


def _augment(mat, bias_row, pad_to):
    """Append [bias_row; zeros] below mat so it has pad_to rows."""
    extra = np.zeros((pad_to - mat.shape[0], mat.shape[1]), np.float32)
    extra[0] = bias_row
    return np.concatenate([mat, extra], axis=0)


def _swizzle_qk(w, EC):
    """[EC*128, D] -> [D//128, 128, EC*128]: per-wave slice partition-major
    so its DMA moves in 4KB packets."""
    D = w.shape[1]
    return np.ascontiguousarray(
        w.reshape(EC, 128, D // 128, 128).transpose(2, 1, 0, 3)
        .reshape(D // 128, 128, EC * 128).astype(BF))


def _swizzle_v(w, EC, VN=512):
    """[EC*128, D] -> [D//VN, 128, EC*VN] partition-major."""
    D = w.shape[1]
    return np.ascontiguousarray(
        w.reshape(EC, 128, D // VN, VN).transpose(2, 1, 0, 3)
        .reshape(D // VN, 128, EC * VN).astype(BF))


_NC_CACHE = {}


def _get_nc(bias):
    if bias not in _NC_CACHE:
        _NC_CACHE[bias] = build_nc(bias=bias)
    return _NC_CACHE[bias]


def kernel(x, Wq, bq, Wk, bk, Wv, bv, Wp, bp):
    global LAST_RESULT
    x = np.ascontiguousarray(np.asarray(x, np.float32))
    Wq, bq = np.asarray(Wq, np.float32), np.asarray(bq, np.float32)
    Wk, bk = np.asarray(Wk, np.float32), np.asarray(bk, np.float32)
    Wv, bv = np.asarray(Wv, np.float32), np.asarray(bv, np.float32)
    Wp, bp = np.asarray(Wp, np.float32), np.asarray(bp, np.float32)

    B, T, C = x.shape
    assert (B, T, C) == (4, 2048, 2048), (B, T, C)
    D = 1024  # head-group width: 8 heads per core
    bias = bool(np.any(bq) or np.any(bk) or np.any(bv))
    nc = _get_nc(bias)

    kk = np.arange(128)[:, None]
    qq = np.arange(128)[None, :]
    tri = (kk <= qq).astype(BF)
    ones = np.ones((128, 128), BF)
    Ep = C + 128 if bias else C

    in_maps = []
    for c in range(N_CORES):
        b, g = c // 2, c % 2
        xt = x[b].T
        wq_g = Wq[:, g * D:(g + 1) * D]
        wk_g = Wk[:, g * D:(g + 1) * D]
        wv_g = Wv[:, g * D:(g + 1) * D]
        if bias:
            xt = _augment(xt, np.ones(T, np.float32), Ep)
            wq_g = _augment(wq_g, bq[g * D:(g + 1) * D], Ep)
            wk_g = _augment(wk_g, bk[g * D:(g + 1) * D], Ep)
            wv_g = _augment(wv_g, bv[g * D:(g + 1) * D], Ep)
        EC = Ep // 128
        in_maps.append({
            "xT": np.ascontiguousarray(xt.astype(BF)),
            "wq": _swizzle_qk(wq_g, EC),
            "wk": _swizzle_qk(wk_g, EC),
            "wv": _swizzle_v(wv_g, EC),
            "wp": np.ascontiguousarray(Wp[g * D:(g + 1) * D, :].astype(BF)),
            "tri": tri,
            "ones": ones,
        })

    trace = bool(os.environ.get("MHA_TRACE"))
    res = run_bass_kernel_spmd(nc, in_maps, core_ids=list(range(N_CORES)),
                               trace=trace)
    LAST_RESULT = res

    out = np.empty((B, T, C), np.float32)
    for b in range(B):
        out[b] = res.results[2 * b]["y"] + res.results[2 * b + 1]["y"]
    out += bp[None, None, :]
    return out


# revision 30
# speedup vs baseline: 1.0111x; 1.0039x over previous
"""TRN2 Bass kernel for 16-head causal MHA (B=4, T=2048, C=2048), fp32 in/out.

Sharding: 8 cores = 4 batches x 2 head-groups (8 heads each).  Each core
computes q/k/v projections for its head group on its batch (tensor-parallel
column split of Wq/Wk/Wv), causal attention in the S^T layout, and a partial
output projection with the row slice of Wp.  The two head-group partials per
batch are summed on the host, plus the output bias.

Design (v6, bf16 datapath):
- All operand data (x^T, Wq/Wk/Wv/Wp, q/k/v, p, atn) is bf16; every matmul
  accumulates in fp32 PSUM.  bf16 stationaries get FWL (fast weight load,
  ~53ns/128-col) so LDWEIGHTS hides under the 512-col moving pass.  Weights
  are host-swizzled so each on-chip slice is partition-contiguous in DRAM
  (4KB DMA packets instead of 256B).
- q^T/k^T ([d,t] layout) and v ([t,d] layout) stay resident in SBUF (12 MB)
  -- no DRAM spill roundtrips between projection and attention.
- Softmax in the S^T layout: scores^T [k,q] per 128-k-chunk; exp on the
  Scalar engine batched over two PSUM banks ([128,1024]) to amortize the
  ~352-cycle ACTIVATE overhead; causal masking is multiplicative (one
  [128,128] lower-triangle bf16 tile on DVE, only on the 4 diagonal 128x128
  sub-blocks per (head, q-group)); strictly-above-diagonal work is skipped,
  and the diagonal blocks are column-trimmed (packed variable-width scores /
  attn matmuls).
- Softmax denominator: p_sum accumulated on DVE in bf16 (2x mode), then ONE
  ones-matmul per (head, q-group) with a [128,128] ones stationary -- the
  matmul itself broadcasts the column-sums to all 128 partitions -- then a
  single-pass DVE reciprocal_approx_fast and a GPSIMD multiply into the
  resident bf16 attention tile.  No DRAM roundtrip.
- Output projection (Wp row-slice) is interleaved per q-group into the
  attention stream one q-group behind, so its PE work fills attention's
  ACT/DVE-bound stretches; y tiles DMA out on the (otherwise idle) Sync
  queue as they finish.
"""
import math
import os
from collections import deque

import ml_dtypes
import numpy as np

import concourse.bass as bass
import concourse.tile as tile
from concourse import bacc, mybir
from concourse.bass_utils import run_bass_kernel_spmd

f32 = mybir.dt.float32
bf16 = mybir.dt.bfloat16
AF = mybir.ActivationFunctionType
BF = ml_dtypes.bfloat16

N_CORES = 8
HD = 128                      # head dim

# results of the last run_bass_kernel_spmd call (for test harness profiling)
LAST_RESULT = None


def build_nc(T=2048, E=2048, D=1024, NOD=2048, TG=512, bias=False,
             num_devices=N_CORES):
    """Build + compile the per-core Bass program."""
    NH = D // HD              # heads per core
    EC = E // 128 + (1 if bias else 0)
    TC = T // 128             # 128-row tiles along T
    TGC = T // TG             # q-groups
    ODG = NOD // 512          # out-proj column groups
    VN = 512                  # v-projection moving width
    NVS = D // VN
    scale = 1.0 / math.sqrt(HD)

    nc = bacc.Bacc("TRN2", target_bir_lowering=False, debug=False,
                   num_devices=num_devices)

    xT_d = nc.dram_tensor("xT", [EC * 128, T], bf16, kind="ExternalInput")
    wq_d = nc.dram_tensor("wq", [D // 128, 128, EC * 128], bf16,
                          kind="ExternalInput")
    wk_d = nc.dram_tensor("wk", [D // 128, 128, EC * 128], bf16,
                          kind="ExternalInput")
    wv_d = nc.dram_tensor("wv", [NVS, 128, EC * VN], bf16,
                          kind="ExternalInput")
    wp_d = nc.dram_tensor("wp", [D, NOD], bf16, kind="ExternalInput")
    tri_d = nc.dram_tensor("tri", [128, 128], bf16, kind="ExternalInput")
    ones_d = nc.dram_tensor("ones", [128, 128], bf16, kind="ExternalInput")
    y_d = nc.dram_tensor("y", [T, NOD], f32, kind="ExternalOutput")

    with tile.TileContext(nc) as tc:
        with tc.tile_pool(name="persist", bufs=1) as persist:
            # q^T/k^T in [d, t] layout (head h = 128-row chunk h), v in
            # natural [t, d] layout ([t%128, tt*D + d]).
            qt_all = persist.tile([128, NH * T], bf16)
            kt_all = persist.tile([128, NH * T], bf16)
            v_all = persist.tile([128, TC * D], bf16)
            ones_sb = persist.tile([128, 128], bf16)
            tri_sb = persist.tile([128, 128], bf16)
            scr = persist.tile([1, 1], f32)
            nc.sync.dma_start(ones_sb[:], ones_d[:])
            nc.sync.dma_start(tri_sb[:], tri_d[:])

            # ---------------- phase A+B: q/k/v projections ----------------
            with (
                tc.tile_pool(name="xt", bufs=1) as xt_pool,
                tc.tile_pool(name="wcola", bufs=3) as wcol_pool,
                tc.tile_pool(name="wvp", bufs=1) as wv_pool,
                tc.tile_pool(name="ab_psum", bufs=8, space="PSUM") as ab_psum,
            ):
                xt_sb = xt_pool.tile([128, EC * T], bf16)

                def xt_e(e):
                    return xt_sb[:, e * T:(e + 1) * T]

                dsts = (qt_all, kt_all)
                wds = (wq_d, wk_d)
                wpairs = [(w_i, dc) for w_i in range(2) for dc in range(D // 128)]

                def load_wcol(w_i, dc):
                    wcol = wcol_pool.tile([128, EC * 128], bf16, tag="wcol",
                                          name=f"wcol_{w_i}_{dc}")
                    nc.sync.dma_start(wcol[:], wds[w_i][dc])
                    return wcol

                # xT spread across THREE DMA queues (gpsimd is idle in this
                # phase; per-queue bandwidth ~180GB/s makes a 0.5MB chunk
                # ~2.8us) so chunk arrival outpaces the first wave pair's
                # e-consumption (~1.7us per chunk); chunk 0 rides gpsimd in
                # parallel with wcol0 on Sync to start the first matmul
                # sooner.
                wcol_q = [load_wcol(*wpairs[0])]
                nc.gpsimd.dma_start(out=xt_sb[:, 0:T], in_=xT_d[0:128, :])
                wcol_q.append(load_wcol(*wpairs[1]))
                for e in range(1, EC):
                    src = xT_d[e * 128:(e + 1) * 128, :]
                    dst = xt_sb[:, e * T:(e + 1) * T]
                    m = e % 3
                    if m == 0:
                        nc.gpsimd.dma_start(out=dst, in_=src)
                    elif m == 1:
                        nc.scalar.dma_start(dst, src)
                    else:
                        nc.sync.dma_start(dst, src)
                # dummy exp (emitted after the xT triggers so it doesn't
                # block them): pulls the ~2.7us ACT table load off phase C's
                # critical path.
                nc.scalar.activation(scr[:], ones_sb[0:1, 0:1], AF.Exp,
                                     scale=1.0)
                wvgs = []
                for dg in range(NVS):
                    wvg = wv_pool.tile([128, EC * VN], bf16, tag=f"wvg{dg}",
                                       name=f"wvg_{dg}")
                    nc.sync.dma_start(wvg[:], wv_d[dg])
                    wvgs.append(wvg)

                # Q^T / K^T: one (weight, d-chunk) per wave of 4 PSUM banks,
                # e-major inside the wave; 8 banks = two waves in flight.
                # The first TWO waves are e-interleaved so each arriving xT
                # chunk feeds 8 matmuls, matching the chunk DMA arrival rate.
                ngrp = T // TG

                def qk_wave_tiles(w_i, dc):
                    return [ab_psum.tile([128, TG], f32, tag="abps",
                                         name=f"abps_{w_i}_{dc}_{tg}")
                            for tg in range(ngrp)]

                def qk_wave_mms(pss, wcol, e):
                    for tg in range(ngrp):
                        nc.tensor.matmul(
                            pss[tg][:],
                            wcol[:, e * 128:(e + 1) * 128],
                            xt_e(e)[:, tg * TG:(tg + 1) * TG],
                            start=(e == 0), stop=(e == EC - 1),
                        )

                def qk_wave_copies(pss, w_i, dc):
                    for tg in range(ngrp):
                        nc.scalar.copy(
                            dsts[w_i][:, dc * T + tg * TG:dc * T + (tg + 1) * TG],
                            pss[tg][:])

                wcol_q.append(load_wcol(*wpairs[2]))
                wcol_q.append(load_wcol(*wpairs[3]))
                pss0 = qk_wave_tiles(*wpairs[0])
                pss1 = qk_wave_tiles(*wpairs[1])
                wcol0, wcol1 = wcol_q.pop(0), wcol_q.pop(0)
                for e in range(EC):
                    qk_wave_mms(pss0, wcol0, e)
                    qk_wave_mms(pss1, wcol1, e)
                qk_wave_copies(pss0, *wpairs[0])
                qk_wave_copies(pss1, *wpairs[1])
                for wi in range(2, len(wpairs)):
                    w_i, dc = wpairs[wi]
                    wcol = wcol_q.pop(0)
                    if wi + 2 < len(wpairs):
                        wcol_q.append(load_wcol(*wpairs[wi + 2]))
                    pss = qk_wave_tiles(w_i, dc)
                    for e in range(EC):
                        qk_wave_mms(pss, wcol, e)
                    qk_wave_copies(pss, w_i, dc)

                # V in natural [t, d] layout at full 512 moving width.
                # Copies alternate ACT/DVE so the tail backlog at the
                # PSUM-pool phase handoff drains twice as fast.
                for tt in range(TC):
                    for dg in range(NVS):
                        ps = ab_psum.tile([128, VN], f32, tag="abps",
                                          name=f"vps_{tt}_{dg}")
                        for e in range(EC):
                            nc.tensor.matmul(
                                ps[:],
                                xt_e(e)[:, tt * 128:(tt + 1) * 128],
                                wvgs[dg][:, e * VN:(e + 1) * VN],
                                start=(e == 0), stop=(e == EC - 1),
                            )
                        dst = v_all[:, tt * D + dg * VN:tt * D + (dg + 1) * VN]
                        if (tt * NVS + dg) % 2:
                            nc.vector.tensor_copy(dst, ps[:])
                        else:
                            nc.scalar.copy(dst, ps[:])

            # ---------------- phase C+D: attention + out-proj ----------------
            with (
                tc.tile_pool(name="cd", bufs=1) as cd_pool,
                tc.tile_pool(name="pt", bufs=3) as pt_pool,
                tc.tile_pool(name="sm", bufs=2) as sm_pool,
                tc.tile_pool(name="s_psum", bufs=2, space="PSUM") as s_psum,
                tc.tile_pool(name="a_psum", bufs=1, space="PSUM") as a_psum,
                tc.tile_pool(name="d_psum", bufs=1, space="PSUM") as d_psum,
                tc.tile_pool(name="y_psum", bufs=2, space="PSUM") as y_psum,
            ):
                atn_all = cd_pool.tile([128, NH * T], bf16)
                wp_sb = cd_pool.tile([128, NH * ODG * 512], bf16)
                nc.scalar.dma_start(
                    wp_sb.rearrange("p (hc og o) -> p hc og o", hc=NH, og=ODG),
                    wp_d.rearrange("(hc p) (og o) -> p hc og o", p=128, o=512),
                )
                dsum_t = d_psum.tile([128, TG], f32)

                def emit_cgroup(qg, h):
                    qbase = qg * TG
                    npairs = 2 * (qg + 1)
                    nk = 4 * (qg + 1)
                    kc0 = qg * 4           # first diagonal k-chunk

                    def pair_desc(p):
                        # [(kc, soff, width, qoff)], exp width
                        if p == npairs - 2:
                            return [(kc0, 0, 512, 0),
                                    (kc0 + 1, 512, 384, 128)], 896
                        if p == npairs - 1:
                            return [(kc0 + 2, 0, 256, 256),
                                    (kc0 + 3, 256, 128, 384)], 384
                        return [(2 * p, 0, 512, 0),
                                (2 * p + 1, 512, 512, 0)], 1024

                    pts = [None] * npairs
                    p_sum = pt_pool.tile([128, TG], bf16, tag="psacc",
                                         bufs=2, name=f"psacc_{qg}_{h}")

                    def emit_av(p):
                        # attn matmuls + DVE p-sum accumulation for pair p
                        # (after its exp/mask)
                        parts, _ = pair_desc(p)
                        p_t = pts[p]
                        for (kc, soff, w, qoff) in parts:
                            nc.tensor.matmul(
                                atn_ps[:, qoff:qoff + w],
                                v_all[:, kc * D + h * HD:kc * D + (h + 1) * HD],
                                p_t[:, soff:soff + w],
                                start=(kc == 0), stop=(kc == nk - 1),
                            )
                        for (kc, soff, w, qoff) in parts:
                            if kc == 0:
                                nc.vector.tensor_copy(p_sum[:], p_t[:, 0:TG])
                            else:
                                nc.vector.tensor_add(
                                    p_sum[:, qoff:qoff + w],
                                    p_sum[:, qoff:qoff + w],
                                    p_t[:, soff:soff + w])

                    atn_ps = a_psum.tile([128, TG], f32, tag="atn",
                                         name=f"atn_{qg}_{h}")
                    for p in range(npairs):
                        parts, expw = pair_desc(p)
                        s_pair = s_psum.tile([128, 2 * TG], f32, tag="sp",
                                             name=f"sp_{qg}_{h}_{p}")
                        # pair B packs both score blocks into one PSUM bank:
                        # exactly one start (bank pending-zero mark) and one
                        # stop per bank.
                        packed = p == npairs - 1
                        for pi, (kc, soff, w, qoff) in enumerate(parts):
                            nc.tensor.matmul(
                                s_pair[:, soff:soff + w],
                                kt_all[:, h * T + kc * 128:h * T + (kc + 1) * 128],
                                qt_all[:, h * T + qbase + qoff:h * T + qbase + 512],
                                start=(not packed or pi == 0),
                                stop=(not packed or pi == len(parts) - 1),
                            )
                        p_t = pt_pool.tile([128, 2 * TG], bf16, tag="pt",
                                           name=f"pt_{qg}_{h}_{p}")
                        pts[p] = p_t
                        nc.scalar.activation(p_t[:, 0:expw], s_pair[:, 0:expw],
                                             AF.Exp, scale=scale)
                        if p >= npairs - 2:
                            # multiplicative causal mask on the two 128-wide
                            # partial-triangle sections of this pair
                            for (kc, soff, w, qoff) in parts:
                                nc.vector.tensor_mul(
                                    p_t[:, soff:soff + 128],
                                    p_t[:, soff:soff + 128],
                                    tri_sb[:])
                        if p > 0:
                            emit_av(p - 1)
                    emit_av(npairs - 1)
                    slot = qg * NH + h
                    atn_u = sm_pool.tile([128, TG], f32, tag="atnu",
                                         name=f"atnu_{slot}")
                    nc.scalar.copy(atn_u[:], atn_ps[:])

                    def finalize():
                        # ones[128,128]-matmul broadcasts the column-sums of
                        # p_sum to every partition; single-pass DVE
                        # reciprocal; gpsimd multiply into atn_all.  Deferred
                        # past the interleaved out-proj blocks so the DVE
                        # p_sum chain drains off the PE critical path.
                        nc.tensor.matmul(dsum_t[:], ones_sb[:], p_sum[:],
                                         start=True, stop=True)
                        recipB = sm_pool.tile([128, TG], f32, tag="rB",
                                              name=f"rB_{slot}")
                        nc.vector.reciprocal_approx_fast(out=recipB[:],
                                                         in_=dsum_t[:])
                        nc.gpsimd.tensor_mul(
                            atn_all[:, h * T + qbase:h * T + qbase + TG],
                            atn_u[:], recipB[:])

                    return finalize

                def emit_dblock(tt, og):
                    ps = y_psum.tile([128, 512], f32, tag="yps",
                                     name=f"yps_{tt}_{og}")
                    for hc in range(NH):
                        nc.tensor.matmul(
                            ps[:],
                            atn_all[:, hc * T + tt * 128:hc * T + (tt + 1) * 128],
                            wp_sb[:, (hc * ODG + og) * 512:(hc * ODG + og + 1) * 512],
                            start=(hc == 0), stop=(hc == NH - 1),
                        )
                    yst = sm_pool.tile([128, 512], f32, tag="yst",
                                       name=f"yst_{tt}_{og}")
                    nc.vector.tensor_copy(yst[:], ps[:])
                    nc.sync.dma_start(
                        y_d[tt * 128:(tt + 1) * 128, og * 512:(og + 1) * 512],
                        yst[:])

                # Ascending q-groups; each group's ACT/DVE-bound stretches
                # are filled with the previous q-group's out-projection
                # blocks; normalization finalizers deferred past them.
                warm_n = [0]

                def emit_warm(n):
                    # consumer-less matmuls into a y-pool tile: PE filler for
                    # the (filler-less) qg=0 stretch so exp-latency stalls
                    # don't cross HAM's idle window and re-throttle the PE.
                    # Later real D-blocks overwrite the tile (start=True).
                    warm_n[0] += 1
                    ps = y_psum.tile([128, 512], f32, tag="yps",
                                     name=f"warm_{warm_n[0]}")
                    for i in range(n):
                        nc.tensor.matmul(ps[:], kt_all[:, 0:128],
                                         qt_all[:, 0:512],
                                         start=(i == 0), stop=(i == n - 1))

                dq = deque()
                for qg in range(TGC):
                    for h in range(NH):
                        fin = emit_cgroup(qg, h)
                        if dq:
                            for _ in range(2):
                                if dq:
                                    emit_dblock(*dq.popleft())
                        else:
                            emit_warm(4)
                        fin()
                    for tt in range(qg * 4, qg * 4 + 4):
                        for og in range(ODG):
                            dq.append((tt, og))
                while dq:
                    emit_dblock(*dq.popleft())

    nc.compile()
    return nc


def _augment(mat, bias_row, pad_to):
    """Append [bias_row; zeros] below mat so it has pad_to rows."""
    extra = np.zeros((pad_to - mat.shape[0], mat.shape[1]), np.float32)
    extra[0] = bias_row
    return np.concatenate([mat, extra], axis=0)


def _swizzle_qk(w, EC):
    """[EC*128, D] -> [D//128, 128, EC*128]: per-wave slice partition-major
    so its DMA moves in 4KB packets."""
    D = w.shape[1]
    return np.ascontiguousarray(
        w.reshape(EC, 128, D // 128, 128).transpose(2, 1, 0, 3)
        .reshape(D // 128, 128, EC * 128).astype(BF))


def _swizzle_v(w, EC, VN=512):
    """[EC*128, D] -> [D//VN, 128, EC*VN] partition-major."""
    D = w.shape[1]
    return np.ascontiguousarray(
        w.reshape(EC, 128, D // VN, VN).transpose(2, 1, 0, 3)
        .reshape(D // VN, 128, EC * VN).astype(BF))


_NC_CACHE = {}


def _get_nc(bias):
    if bias not in _NC_CACHE:
        _NC_CACHE[bias] = build_nc(bias=bias)
    return _NC_CACHE[bias]


def kernel(x, Wq, bq, Wk, bk, Wv, bv, Wp, bp):
    global LAST_RESULT
    x = np.ascontiguousarray(np.asarray(x, np.float32))
    Wq, bq = np.asarray(Wq, np.float32), np.asarray(bq, np.float32)
    Wk, bk = np.asarray(Wk, np.float32), np.asarray(bk, np.float32)
    Wv, bv = np.asarray(Wv, np.float32), np.asarray(bv, np.float32)
    Wp, bp = np.asarray(Wp, np.float32), np.asarray(bp, np.float32)

    B, T, C = x.shape
    assert (B, T, C) == (4, 2048, 2048), (B, T, C)
    D = 1024  # head-group width: 8 heads per core
    bias = bool(np.any(bq) or np.any(bk) or np.any(bv))
    nc = _get_nc(bias)

    kk = np.arange(128)[:, None]
    qq = np.arange(128)[None, :]
    tri = (kk <= qq).astype(BF)
    ones = np.ones((128, 128), BF)
    Ep = C + 128 if bias else C

    in_maps = []
    for c in range(N_CORES):
        b, g = c // 2, c % 2
        xt = x[b].T
        wq_g = Wq[:, g * D:(g + 1) * D]
        wk_g = Wk[:, g * D:(g + 1) * D]
        wv_g = Wv[:, g * D:(g + 1) * D]
        if bias:
            xt = _augment(xt, np.ones(T, np.float32), Ep)
            wq_g = _augment(wq_g, bq[g * D:(g + 1) * D], Ep)
            wk_g = _augment(wk_g, bk[g * D:(g + 1) * D], Ep)
            wv_g = _augment(wv_g, bv[g * D:(g + 1) * D], Ep)
        EC = Ep // 128
        in_maps.append({
            "xT": np.ascontiguousarray(xt.astype(BF)),
            "wq": _swizzle_qk(wq_g, EC),
            "wk": _swizzle_qk(wk_g, EC),
            "wv": _swizzle_v(wv_g, EC),
            "wp": np.ascontiguousarray(Wp[g * D:(g + 1) * D, :].astype(BF)),
            "tri": tri,
            "ones": ones,
        })

    trace = bool(os.environ.get("MHA_TRACE"))
    res = run_bass_kernel_spmd(nc, in_maps, core_ids=list(range(N_CORES)),
                               trace=trace)
    LAST_RESULT = res

    out = np.empty((B, T, C), np.float32)
    for b in range(B):
        out[b] = res.results[2 * b]["y"] + res.results[2 * b + 1]["y"]
    out += bp[None, None, :]
    return out
